# revision 6
# baseline (speedup 1.0000x reference)
"""Trainium2 Bass kernel for a 5x5 valid convolution over 96x96 images.

Reference computes x @ W.T where W is the [8464, 9216] conv-as-matmul
matrix (10 GFLOP dense).  We instead compute the convolution directly on
the tensor engine as 5 PSUM-accumulated banded matmuls (row-conv over the
image-row contraction, column shifts folded into the rhs access pattern):

    out[oi, b, oj] = sum_kj  B_kj.T @ X[:, b, oj+kj]
    B_kj[i, oi]    = K[i-oi, kj]   (banded Toeplitz, built on device)

Sharding: data-parallel over batch; each of the 8 cores convolves 8
images.  Raw Bass (no TileContext) with a hand-scheduled static DAG:

  sync:    x load ............................ y stores (per half)
  scalar:  zero u | scatter taps k->u | banded u->B_tmp load
  vector:  memset | B reversal | psum->sbuf copies (per half)
  tensor:  HAM warmup matmuls | 2x5 accumulated conv matmuls
"""

import sys

sys.path.insert(0, "/opt/trn_rl_repo")

import numpy as np

import bass_rust
import concourse.bass as bass
import concourse.mybir as mybir
from concourse.bass_utils import run_bass_kernel_spmd

# Problem geometry (hardcoded per the task contract).
BATCH = 64
IN = 96           # input image side
KD = 5            # conv kernel side
OD = IN - KD + 1  # output side = 92
ISIZE = IN * IN   # 9216
OSIZE = OD * OD   # 8464
NCORES = 8
BPC = BATCH // NCORES  # images per core = 8
HALF = BPC // 2        # images per PSUM accumulation group = 4
UL = 187               # per-kj stripe length in the padded tap vector u


def _ap(view, offset, dims):
    ap = view.copy()
    ap.offset = offset
    ap.ap = bass_rust.VecI64Pair(dims)
    return ap


def _build_program(warmup_mms=16, warmup_n=32):
    nc = bass.Bass()
    dt = mybir.dt.float32

    x_in = nc.declare_dram_parameter("x", [BPC, ISIZE], dt, isOutput=False)
    k_in = nc.declare_dram_parameter("k", [KD, KD], dt, isOutput=False)
    y_out = nc.declare_dram_parameter("y", [BPC, OSIZE], dt, isOutput=True)
    u_dram = nc.dram_tensor("u_scratch", [KD * UL], dt)

    with (
        nc.sbuf_tensor([1, KD * UL], dt) as z_sb,
        nc.sbuf_tensor([IN, KD, OD], dt) as b_tmp,
        nc.sbuf_tensor([IN, KD, OD], dt) as b_sb,
        nc.sbuf_tensor([IN, BPC, IN], dt) as x_sb,
        nc.sbuf_tensor([OD, BPC, OD], dt) as out_sb,
        nc.sbuf_tensor([128, 128], dt) as junk_sb,
        nc.psum_tensor([OD, HALF, OD], dt) as ps0,
        nc.psum_tensor([OD, HALF, OD], dt) as ps1,
        nc.psum_tensor([128, 128], dt) as ps_junk,
        nc.semaphore() as sem_z,      # z_sb memset done
        nc.semaphore() as sem_x,      # x -> x_sb
        nc.semaphore() as sem_zout,   # z_sb -> u_dram
        nc.semaphore() as sem_scat,   # taps k_in -> u_dram
        nc.semaphore() as sem_btmp,   # u_dram -> b_tmp
        nc.semaphore() as sem_brev,   # b_tmp reversed -> b_sb
        nc.semaphore() as sem_mm,     # psum group done
        nc.semaphore() as sem_copy,   # psum -> out_sb half done
        nc.semaphore() as sem_y,      # out_sb -> y
        nc.Block(no_gpsimd_drain=True) as block,
    ):
        psums = [ps0, ps1]

        @block.sync
        def _(sync):
            sync.dma_start(
                out=x_sb[:],
                in_=_ap(x_in[:], 0, [[IN, IN], [ISIZE, BPC], [1, IN]]),
            ).then_inc(sem_x, 16)
            # y[b, oi*92+oj] <- out_sb[oi, b, oj], one half at a time
            for h in range(2):
                sync.wait_ge(sem_copy, h + 1)
                sync.dma_start(
                    out=_ap(
                        y_out[:],
                        h * HALF * OSIZE,
                        [[OD, OD], [OSIZE, HALF], [1, OD]],
                    ),
                    in_=out_sb[:, h * HALF : (h + 1) * HALF, :],
                ).then_inc(sem_y, 16)
            sync.wait_ge(sem_y, 32)

        @block.scalar
        def _(scalar):
            scalar.wait_ge(sem_z, 1)
            scalar.dma_start(
                out=_ap(u_dram[:], 0, [[KD * UL, 1], [1, KD * UL]]), in_=z_sb[:]
            ).then_inc(sem_zout, 16)
            scalar.wait_ge(sem_zout, 16)
            # u[kj*UL + 91 + t] = K[t, kj]   (DRAM -> DRAM scatter, 25 elems)
            with nc.allow_non_contiguous_dma(reason="25-element tap scatter"):
                scalar.dma_start(
                    out=_ap(u_dram[:], OD - 1, [[UL, KD], [1, KD]]),
                    in_=_ap(k_in[:], 0, [[1, KD], [KD, KD]]),
                ).then_inc(sem_scat, 16)
            scalar.wait_ge(sem_scat, 16)
            # B_tmp[p, kj, r] = u[kj*UL + p + r]  (= B[p, kj, 91-r])
            scalar.dma_start(
                out=b_tmp[:],
                in_=_ap(u_dram[:], 0, [[1, IN], [UL, KD], [1, OD]]),
            ).then_inc(sem_btmp, 16)

        @block.vector
        def _(vector):
            nc.vector.memset(z_sb[:], 0.0).then_inc(sem_z, 1)
            vector.wait_ge(sem_btmp, 16)
            # reverse the oi axis: B[p, kj, oi] = B_tmp[p, kj, 91-oi]
            nc.vector.tensor_copy(
                b_sb[:],
                _ap(b_tmp[:], OD - 1, [[KD * OD, IN], [OD, KD], [-1, OD]]),
            ).then_inc(sem_brev, 1)
            for h in range(2):
                vector.wait_ge(sem_mm, h + 1)
                nc.vector.tensor_copy(
                    out_sb[:, h * HALF : (h + 1) * HALF, :], psums[h][:]
                ).then_inc(sem_copy, 1)

        @block.tensor
        def _(tensor):
            # HAM warmup: keep the PE busy while DMAs/B-build run so the
            # real matmuls execute at 2.4 GHz instead of 1.2 GHz.
            for _ in range(warmup_mms):
                nc.tensor.matmul(
                    ps_junk[:, :warmup_n],
                    junk_sb[:],
                    junk_sb[:, :warmup_n],
                    start=True,
                    stop=True,
                )
            tensor.wait_ge(sem_brev, 1)
            tensor.wait_ge(sem_x, 16)
            for h in range(2):
                for kj in range(KD):
                    mm = nc.tensor.matmul(
                        psums[h][:],
                        b_sb[:, kj, :],
                        _ap(
                            x_sb[:],
                            h * HALF * IN + kj,
                            [[BPC * IN, IN], [IN, HALF], [1, OD]],
                        ),
                        start=(kj == 0),
                        stop=(kj == KD - 1),
                    )
                    if kj == KD - 1:
                        mm.then_inc(sem_mm, 1)

    return nc


_NC = None


def kernel(x: np.ndarray, kernel: np.ndarray) -> np.ndarray:
    global _NC
    if _NC is None:
        _NC = _build_program()

    x = np.ascontiguousarray(x, dtype=np.float32)
    k = np.ascontiguousarray(kernel, dtype=np.float32)
    in_maps = [
        {"x": x[c * BPC : (c + 1) * BPC], "k": k} for c in range(NCORES)
    ]
    res = run_bass_kernel_spmd(_NC, in_maps, list(range(NCORES)))
    return np.concatenate([res.results[c]["y"] for c in range(NCORES)], axis=0)


# revision 10
# speedup vs baseline: 1.1041x; 1.1041x over previous
"""Trainium2 Bass kernel for a 5x5 valid convolution over 96x96 images.

Reference computes x @ W.T where W is the [8464, 9216] conv-as-matmul
matrix (10 GFLOP dense).  We instead compute the convolution directly on
the tensor engine as 5 PSUM-accumulated banded matmuls (row-conv over the
image-row contraction, column shifts folded into the rhs access pattern):

    out[oi, b, oj] = sum_kj  B_kj.T @ X[:, b, oj+kj]
    B_kj[i, oi]    = K[i-oi, kj]   (banded Toeplitz, built on device)

Sharding: data-parallel over batch; each of the 8 cores convolves 8
images.  Raw Bass (no TileContext) with a hand-scheduled static DAG:

  sync:    x load ............................ y stores (per half)
  scalar:  zero u | scatter taps k->u | banded u->B_tmp load
  vector:  memset | B reversal | psum->sbuf copies (per half)
  tensor:  HAM warmup matmuls | 2x5 accumulated conv matmuls
"""

import sys

sys.path.insert(0, "/opt/trn_rl_repo")

import numpy as np

import bass_rust
import concourse.bass as bass
import concourse.mybir as mybir
from concourse.bass_utils import run_bass_kernel_spmd

# Problem geometry (hardcoded per the task contract).
BATCH = 64
IN = 96           # input image side
KD = 5            # conv kernel side
OD = IN - KD + 1  # output side = 92
ISIZE = IN * IN   # 9216
OSIZE = OD * OD   # 8464
NCORES = 8
BPC = BATCH // NCORES  # images per core = 8
HALF = BPC // 2        # images per PSUM accumulation group = 4
UL = 187               # per-kj stripe length in the padded tap vector u


def _ap(view, offset, dims):
    ap = view.copy()
    ap.offset = offset
    ap.ap = bass_rust.VecI64Pair(dims)
    return ap


def _build_program(warmup_mms=16, warmup_n=32):
    nc = bass.Bass()
    dt = mybir.dt.float32

    x_in = nc.declare_dram_parameter("x", [BPC, ISIZE], dt, isOutput=False)
    k_in = nc.declare_dram_parameter("k", [KD, KD], dt, isOutput=False)
    y_out = nc.declare_dram_parameter("y", [BPC, OSIZE], dt, isOutput=True)
    # Zero-initialized at NEFF load; per-run the scatter below overwrites
    # all 25 tap positions, so repeated executions stay correct.
    u_dram = nc.inline_tensor(np.zeros(KD * UL, np.float32), "u_scratch")

    with (
        nc.sbuf_tensor([IN, KD, OD], dt) as b_tmp,
        nc.sbuf_tensor([IN, KD, OD], dt) as b_sb,
        nc.sbuf_tensor([IN, BPC, IN], dt) as x_sb,
        nc.sbuf_tensor([OD, BPC, OD], dt) as out_sb,
        nc.sbuf_tensor([128, 128], dt) as junk_sb,
        nc.psum_tensor([OD, HALF, OD], dt) as ps0,
        nc.psum_tensor([OD, HALF, OD], dt) as ps1,
        nc.psum_tensor([128, 128], dt) as ps_junk,
        nc.semaphore() as sem_x,      # x -> x_sb
        nc.semaphore() as sem_scat,   # taps k_in -> u_dram
        nc.semaphore() as sem_btmp,   # u_dram -> b_tmp
        nc.semaphore() as sem_brev,   # b_tmp reversed -> b_sb
        nc.semaphore() as sem_mm,     # psum group done
        nc.semaphore() as sem_copy,   # psum -> out_sb half done
        nc.semaphore() as sem_y,      # out_sb -> y
        nc.Block(no_gpsimd_drain=True) as block,
    ):
        psums = [ps0, ps1]

        @block.sync
        def _(sync):
            sync.dma_start(
                out=x_sb[:],
                in_=_ap(x_in[:], 0, [[IN, IN], [ISIZE, BPC], [1, IN]]),
            ).then_inc(sem_x, 16)
            # y[b, oi*92+oj] <- out_sb[oi, b, oj], one half at a time
            for h in range(2):
                sync.wait_ge(sem_copy, h + 1)
                sync.dma_start(
                    out=_ap(
                        y_out[:],
                        h * HALF * OSIZE,
                        [[OD, OD], [OSIZE, HALF], [1, OD]],
                    ),
                    in_=out_sb[:, h * HALF : (h + 1) * HALF, :],
                ).then_inc(sem_y, 16)
            sync.wait_ge(sem_y, 32)

        @block.scalar
        def _(scalar):
            # u[kj*UL + 91 + t] = K[t, kj]   (DRAM -> DRAM scatter, 25 elems)
            with nc.allow_non_contiguous_dma(reason="25-element tap scatter"):
                scalar.dma_start(
                    out=_ap(u_dram[:], OD - 1, [[UL, KD], [1, KD]]),
                    in_=_ap(k_in[:], 0, [[1, KD], [KD, KD]]),
                ).then_inc(sem_scat, 16)
            scalar.wait_ge(sem_scat, 16)
            # B_tmp[p, kj, r] = u[kj*UL + p + r]  (= B[p, kj, 91-r])
            scalar.dma_start(
                out=b_tmp[:],
                in_=_ap(u_dram[:], 0, [[1, IN], [UL, KD], [1, OD]]),
            ).then_inc(sem_btmp, 16)

        @block.vector
        def _(vector):
            vector.wait_ge(sem_btmp, 16)
            # reverse the oi axis: B[p, kj, oi] = B_tmp[p, kj, 91-oi]
            nc.vector.tensor_copy(
                b_sb[:],
                _ap(b_tmp[:], OD - 1, [[KD * OD, IN], [OD, KD], [-1, OD]]),
            ).then_inc(sem_brev, 1)
            for h in range(2):
                vector.wait_ge(sem_mm, h + 1)
                nc.vector.tensor_copy(
                    out_sb[:, h * HALF : (h + 1) * HALF, :], psums[h][:]
                ).then_inc(sem_copy, 1)

        @block.tensor
        def _(tensor):
            # HAM warmup: keep the PE busy while DMAs/B-build run so the
            # real matmuls execute at 2.4 GHz instead of 1.2 GHz.
            for _ in range(warmup_mms):
                nc.tensor.matmul(
                    ps_junk[:, :warmup_n],
                    junk_sb[:],
                    junk_sb[:, :warmup_n],
                    start=True,
                    stop=True,
                )
            tensor.wait_ge(sem_brev, 1)
            tensor.wait_ge(sem_x, 16)
            for h in range(2):
                for kj in range(KD):
                    mm = nc.tensor.matmul(
                        psums[h][:],
                        b_sb[:, kj, :],
                        _ap(
                            x_sb[:],
                            h * HALF * IN + kj,
                            [[BPC * IN, IN], [IN, HALF], [1, OD]],
                        ),
                        start=(kj == 0),
                        stop=(kj == KD - 1),
                    )
                    if kj == KD - 1:
                        mm.then_inc(sem_mm, 1)

    return nc


_NC = None


def kernel(x: np.ndarray, kernel: np.ndarray) -> np.ndarray:
    global _NC
    if _NC is None:
        _NC = _build_program()

    x = np.ascontiguousarray(x, dtype=np.float32)
    k = np.ascontiguousarray(kernel, dtype=np.float32)
    in_maps = [
        {"x": x[c * BPC : (c + 1) * BPC], "k": k} for c in range(NCORES)
    ]
    res = run_bass_kernel_spmd(_NC, in_maps, list(range(NCORES)))
    return np.concatenate([res.results[c]["y"] for c in range(NCORES)], axis=0)


# revision 14
# speedup vs baseline: 1.2681x; 1.1485x over previous
"""Trainium2 Bass kernel for a 5x5 valid convolution over 96x96 images.

Reference computes x @ W.T where W is the [8464, 9216] conv-as-matmul
matrix (10 GFLOP dense).  We instead compute the convolution directly on
the tensor engine as 5 PSUM-accumulated banded matmuls (row-conv over the
image-row contraction, column shifts folded into the rhs access pattern):

    out[oi, b, oj] = sum_kj  B_kj.T @ X[:, b, oj+kj]
    B_kj[i, oi]    = K[i-oi, kj]   (banded Toeplitz, built on device)

Sharding: data-parallel over batch; each of the 8 cores convolves 8
images.  Raw Bass (no TileContext) with a hand-scheduled static DAG:

  sync:    x load ............................ y stores (per half)
  scalar:  zero u | scatter taps k->u | banded u->B_tmp load
  vector:  memset | B reversal | psum->sbuf copies (per half)
  tensor:  HAM warmup matmuls | 2x5 accumulated conv matmuls
"""

import sys

sys.path.insert(0, "/opt/trn_rl_repo")

import numpy as np

import bass_rust
import concourse.bass as bass
import concourse.mybir as mybir
from concourse.bass_utils import run_bass_kernel_spmd

# Problem geometry (hardcoded per the task contract).
BATCH = 64
IN = 96           # input image side
KD = 5            # conv kernel side
OD = IN - KD + 1  # output side = 92
ISIZE = IN * IN   # 9216
OSIZE = OD * OD   # 8464
NCORES = 8
BPC = BATCH // NCORES  # images per core = 8
HALF = BPC // 2        # images per PSUM accumulation group = 4
UL = 187               # per-kj stripe length in the padded tap vector u


def _ap(view, offset, dims):
    ap = view.copy()
    ap.offset = offset
    ap.ap = bass_rust.VecI64Pair(dims)
    return ap


def _build_program(warmup_mms=16, warmup_n=32):
    nc = bass.Bass()
    dt = mybir.dt.float32

    x_in = nc.declare_dram_parameter("x", [BPC, ISIZE], dt, isOutput=False)
    k_in = nc.declare_dram_parameter("k", [KD, KD], dt, isOutput=False)
    y_out = nc.declare_dram_parameter("y", [BPC, OSIZE], dt, isOutput=True)
    # Zero-initialized at NEFF load; per-run the scatter below overwrites
    # all 25 tap positions, so repeated executions stay correct.
    u_dram = nc.inline_tensor(np.zeros(KD * UL, np.float32), "u_scratch")

    f32r = mybir.dt.float32r
    with (
        nc.sbuf_tensor([IN, KD, OD], dt) as b_tmp,
        nc.sbuf_tensor([IN, KD, OD], f32r) as b_sb,
        nc.sbuf_tensor([IN, BPC, IN], dt) as x_sb,
        nc.sbuf_tensor([IN, BPC, IN], f32r) as x_r,
        nc.sbuf_tensor([OD, BPC, OD], dt) as out_sb,
        nc.sbuf_tensor([128, 128], dt) as junk_sb,
        nc.psum_tensor([OD, HALF, OD], dt) as ps0,
        nc.psum_tensor([OD, HALF, OD], dt) as ps1,
        nc.psum_tensor([128, 128], dt) as ps_junk,
        nc.semaphore() as sem_x,      # x -> x_sb
        nc.semaphore() as sem_xr,     # x rounded to f32r
        nc.semaphore() as sem_scat,   # taps k_in -> u_dram
        nc.semaphore() as sem_btmp,   # u_dram -> b_tmp
        nc.semaphore() as sem_brev,   # b_tmp reversed -> b_sb
        nc.semaphore() as sem_mm,     # psum group done
        nc.semaphore() as sem_copy,   # psum -> out_sb half done
        nc.semaphore() as sem_y,      # out_sb -> y
        nc.Block(no_gpsimd_drain=True) as block,
    ):
        psums = [ps0, ps1]

        @block.sync
        def _(sync):
            sync.dma_start(
                out=x_sb[:],
                in_=_ap(x_in[:], 0, [[IN, IN], [ISIZE, BPC], [1, IN]]),
            ).then_inc(sem_x, 16)
            # y[b, oi*92+oj] <- out_sb[oi, b, oj], one half at a time
            for h in range(2):
                sync.wait_ge(sem_copy, h + 1)
                sync.dma_start(
                    out=_ap(
                        y_out[:],
                        h * HALF * OSIZE,
                        [[OD, OD], [OSIZE, HALF], [1, OD]],
                    ),
                    in_=out_sb[:, h * HALF : (h + 1) * HALF, :],
                ).then_inc(sem_y, 16)
            sync.wait_ge(sem_y, 32)

        @block.scalar
        def _(scalar):
            # u[kj*UL + 91 + t] = K[t, kj]   (DRAM -> DRAM scatter, 25 elems)
            with nc.allow_non_contiguous_dma(reason="25-element tap scatter"):
                scalar.dma_start(
                    out=_ap(u_dram[:], OD - 1, [[UL, KD], [1, KD]]),
                    in_=_ap(k_in[:], 0, [[1, KD], [KD, KD]]),
                ).then_inc(sem_scat, 16)
            scalar.wait_ge(sem_scat, 16)
            # B_tmp[p, kj, r] = u[kj*UL + p + r]  (= B[p, kj, 91-r])
            scalar.dma_start(
                out=b_tmp[:],
                in_=_ap(u_dram[:], 0, [[1, IN], [UL, KD], [1, OD]]),
            ).then_inc(sem_btmp, 16)

        @block.vector
        def _(vector):
            # round x to f32r for the 1-cycle/row matmul path (DVE is idle
            # here; x_r is ready well before B)
            vector.wait_ge(sem_x, 16)
            nc.vector.tensor_copy(x_r[:], x_sb[:]).then_inc(sem_xr, 1)
            vector.wait_ge(sem_btmp, 16)
            # reverse the oi axis: B[p, kj, oi] = B_tmp[p, kj, 91-oi]
            # (the f32r-dtype output also performs the required rounding)
            nc.vector.tensor_copy(
                b_sb[:],
                _ap(b_tmp[:], OD - 1, [[KD * OD, IN], [OD, KD], [-1, OD]]),
            ).then_inc(sem_brev, 1)
            for h in range(2):
                vector.wait_ge(sem_mm, h + 1)
                nc.vector.tensor_copy(
                    out_sb[:, h * HALF : (h + 1) * HALF, :], psums[h][:]
                ).then_inc(sem_copy, 1)

        @block.tensor
        def _(tensor):
            # HAM warmup: keep the PE busy while DMAs/B-build run so the
            # real matmuls execute at 2.4 GHz instead of 1.2 GHz.
            for _ in range(warmup_mms):
                nc.tensor.matmul(
                    ps_junk[:, :warmup_n],
                    junk_sb[:],
                    junk_sb[:, :warmup_n],
                    start=True,
                    stop=True,
                )
            tensor.wait_ge(sem_brev, 1)
            tensor.wait_ge(sem_xr, 1)
            for h in range(2):
                for kj in range(KD):
                    mm = nc.tensor.matmul(
                        psums[h][:],
                        b_sb[:, kj, :],
                        _ap(
                            x_r[:],
                            h * HALF * IN + kj,
                            [[BPC * IN, IN], [IN, HALF], [1, OD]],
                        ),
                        start=(kj == 0),
                        stop=(kj == KD - 1),
                    )
                    if kj == KD - 1:
                        mm.then_inc(sem_mm, 1)

    return nc


_NC = None


def kernel(x: np.ndarray, kernel: np.ndarray) -> np.ndarray:
    global _NC
    if _NC is None:
        _NC = _build_program()

    x = np.ascontiguousarray(x, dtype=np.float32)
    k = np.ascontiguousarray(kernel, dtype=np.float32)
    in_maps = [
        {"x": x[c * BPC : (c + 1) * BPC], "k": k} for c in range(NCORES)
    ]
    res = run_bass_kernel_spmd(_NC, in_maps, list(range(NCORES)))
    return np.concatenate([res.results[c]["y"] for c in range(NCORES)], axis=0)


# revision 15
# speedup vs baseline: 1.3634x; 1.0752x over previous
"""Trainium2 Bass kernel for a 5x5 valid convolution over 96x96 images.

Reference computes x @ W.T where W is the [8464, 9216] conv-as-matmul
matrix (10 GFLOP dense).  We instead compute the convolution directly on
the tensor engine as 5 PSUM-accumulated banded matmuls (row-conv over the
image-row contraction, column shifts folded into the rhs access pattern):

    out[oi, b, oj] = sum_kj  B_kj.T @ X[:, b, oj+kj]
    B_kj[i, oi]    = K[i-oi, kj]   (banded Toeplitz, built on device)

Sharding: data-parallel over batch; each of the 8 cores convolves 8
images.  Raw Bass without a Block (walrus's own end-of-stream handshake
replaces the explicit all-engine barrier), hand-scheduled static DAG:

  sync:    x load | y quarter-stores q0,q2 | final completion wait
  scalar:  scatter taps k->u | banded u->B_tmp load | y stores q1,q3
  vector:  x->f32r round | B reversal | psum->sbuf quarter copies
  tensor:  2x5 accumulated f32r conv matmuls
"""

import sys

sys.path.insert(0, "/opt/trn_rl_repo")

import numpy as np

import bass_rust
import concourse.bass as bass
import concourse.mybir as mybir
from concourse.bass_utils import run_bass_kernel_spmd

# Problem geometry (hardcoded per the task contract).
BATCH = 64
IN = 96           # input image side
KD = 5            # conv kernel side
OD = IN - KD + 1  # output side = 92
ISIZE = IN * IN   # 9216
OSIZE = OD * OD   # 8464
NCORES = 8
BPC = BATCH // NCORES  # images per core = 8
HALF = BPC // 2        # images per PSUM accumulation group = 4
QTR = BPC // 4         # images per store quarter = 2
UL = 187               # per-kj stripe length in the padded tap vector u


def _ap(view, offset, dims):
    ap = view.copy()
    ap.offset = offset
    ap.ap = bass_rust.VecI64Pair(dims)
    return ap


def _build_program():
    nc = bass.Bass()
    dt = mybir.dt.float32
    f32r = mybir.dt.float32r

    x_in = nc.declare_dram_parameter("x", [BPC, ISIZE], dt, isOutput=False)
    k_in = nc.declare_dram_parameter("k", [KD, KD], dt, isOutput=False)
    y_out = nc.declare_dram_parameter("y", [BPC, OSIZE], dt, isOutput=True)
    # Zero-initialized at NEFF load; per-run the scatter below overwrites
    # all 25 tap positions, so repeated executions stay correct.
    u_dram = nc.inline_tensor(np.zeros(KD * UL, np.float32), "u_scratch")

    with (
        nc.sbuf_tensor([IN, KD, OD], dt) as b_tmp,
        nc.sbuf_tensor([IN, KD, OD], f32r) as b_sb,
        nc.sbuf_tensor([IN, BPC, IN], dt) as x_sb,
        nc.sbuf_tensor([IN, BPC, IN], f32r) as x_r,
        nc.sbuf_tensor([OD, BPC, OD], dt) as out_sb,
        nc.psum_tensor([OD, HALF, OD], dt) as ps0,
        nc.psum_tensor([OD, HALF, OD], dt) as ps1,
        nc.semaphore() as sem_x,      # x -> x_sb
        nc.semaphore() as sem_xr,     # x rounded to f32r
        nc.semaphore() as sem_scat,   # taps k_in -> u_dram
        nc.semaphore() as sem_btmp,   # u_dram -> b_tmp
        nc.semaphore() as sem_brev,   # b_tmp reversed -> b_sb
        nc.semaphore() as sem_mm,     # psum group done
        nc.semaphore() as sem_copy,   # psum -> out_sb quarter done
        nc.semaphore() as sem_y,      # out_sb -> y
    ):
        psums = [ps0, ps1]

        # ---- scalar (ACT HWDGE ring): B build chain, then stores q1,q3
        # u[kj*UL + 91 + t] = K[t, kj]   (DRAM -> DRAM scatter, 25 elems)
        with nc.allow_non_contiguous_dma(reason="25-element tap scatter"):
            nc.scalar.dma_start(
                out=_ap(u_dram[:], OD - 1, [[UL, KD], [1, KD]]),
                in_=_ap(k_in[:], 0, [[1, KD], [KD, KD]]),
            ).then_inc(sem_scat, 16)
        nc.scalar.wait_ge(sem_scat, 16)
        # B_tmp[p, kj, r] = u[kj*UL + p + r]  (= B[p, kj, 91-r])
        nc.scalar.dma_start(
            out=b_tmp[:],
            in_=_ap(u_dram[:], 0, [[1, IN], [UL, KD], [1, OD]]),
        ).then_inc(sem_btmp, 16)

        # ---- sync (SP HWDGE ring): x load
        nc.sync.dma_start(
            out=x_sb[:],
            in_=_ap(x_in[:], 0, [[IN, IN], [ISIZE, BPC], [1, IN]]),
        ).then_inc(sem_x, 16)

        # ---- vector: f32r rounding of x, B reversal, psum copies
        nc.vector.wait_ge(sem_x, 16)
        nc.vector.tensor_copy(x_r[:], x_sb[:]).then_inc(sem_xr, 1)
        nc.vector.wait_ge(sem_btmp, 16)
        # reverse the oi axis: B[p, kj, oi] = B_tmp[p, kj, 91-oi]
        # (the f32r output dtype also performs the required rounding)
        nc.vector.tensor_copy(
            b_sb[:],
            _ap(b_tmp[:], OD - 1, [[KD * OD, IN], [OD, KD], [-1, OD]]),
        ).then_inc(sem_brev, 1)

        # ---- tensor: 2 halves x 5 kj accumulated f32r matmuls
        nc.tensor.wait_ge(sem_brev, 1)
        nc.tensor.wait_ge(sem_xr, 1)
        for h in range(2):
            for kj in range(KD):
                mm = nc.tensor.matmul(
                    psums[h][:],
                    b_sb[:, kj, :],
                    _ap(
                        x_r[:],
                        h * HALF * IN + kj,
                        [[BPC * IN, IN], [IN, HALF], [1, OD]],
                    ),
                    start=(kj == 0),
                    stop=(kj == KD - 1),
                )
                if kj == KD - 1:
                    mm.then_inc(sem_mm, 1)

        # ---- vector: quarter copies psum -> out_sb (q covers images 2q..2q+1)
        for q in range(4):
            h, lo = q // 2, (q % 2) * QTR
            nc.vector.wait_ge(sem_mm, h + 1)
            nc.vector.tensor_copy(
                out_sb[:, q * QTR : (q + 1) * QTR, :],
                psums[h][:, lo : lo + QTR, :],
            ).then_inc(sem_copy, 1)

        # ---- stores: quarters alternate between the two HWDGE rings
        def store(engine, q):
            engine.wait_ge(sem_copy, q + 1)
            engine.dma_start(
                out=_ap(
                    y_out[:],
                    q * QTR * OSIZE,
                    [[OD, OD], [OSIZE, QTR], [1, OD]],
                ),
                in_=out_sb[:, q * QTR : (q + 1) * QTR, :],
            ).then_inc(sem_y, 16)

        store(nc.sync, 0)
        store(nc.scalar, 1)
        store(nc.sync, 2)
        store(nc.scalar, 3)
        # hold execution open until every store has landed
        nc.sync.wait_ge(sem_y, 64)

    return nc


_NC = None


def kernel(x: np.ndarray, kernel: np.ndarray) -> np.ndarray:
    global _NC
    if _NC is None:
        _NC = _build_program()

    x = np.ascontiguousarray(x, dtype=np.float32)
    k = np.ascontiguousarray(kernel, dtype=np.float32)
    in_maps = [
        {"x": x[c * BPC : (c + 1) * BPC], "k": k} for c in range(NCORES)
    ]
    res = run_bass_kernel_spmd(_NC, in_maps, list(range(NCORES)))
    return np.concatenate([res.results[c]["y"] for c in range(NCORES)], axis=0)
